# revision 3
# baseline (speedup 1.0000x reference)
"""Trainium2 Bass kernel for nn_FKRM_85839216378385 (vq_codebook).

Strategy (8 NeuronCores, SPMD):
  - Attention branch ([B*HW, n_embed] softmax-attention over an 8192-entry
    codebook) is sharded over PIXELS: core i handles image rows
    [12*i, 12*i+12) of both batches = 2304 pixels, attending over the full
    codebook. The softmax is fused (never materialized in HBM): scores^T are
    built codebook-major ([128 codes x pix] tiles) with 4 row-group-packed
    K=3 matmuls running concurrently in the PE array, exp'd on the scalar
    engine straight out of PSUM, and contracted with v_ext = [v | 1] so the
    softmax numerator and denominator come out of one PSUM accumulation.
  - The PSF image-fusion branch needs global per-batch min/max of the cosine
    map, so it is replicated on every core (it is tiny); its contribution to
    the fused output conv (channels 3..5 of fuse_w) is emitted as a separate
    full-image output `ff_full`, and the attention contribution (channels
    0..2 + bias) as a per-core slice `out_a`. The host adds them.
  - All PSF square roots run as fast-inverse-sqrt + 2 Newton steps on the
    vector engine, so the scalar engine's activation table never leaves the
    natural_log_exp set during the hot exp loop (layernorm rstd uses
    exp(-0.5*ln(v+eps)) from the same set).
  - Weight-only transforms (k = k_w @ bg_embed etc.) are repacked on host.
"""

import numpy as np

N_CORES = 8
B, C, H, W = 2, 3, 96, 96
D = 3
NE = 8192
NWIN = 7
PAD = NWIN // 2          # 3
WP = W + 2 * PAD         # 102
HSL = H // N_CORES       # 12 rows per core (per batch)
PIX = B * HSL * W        # 2304 pixels per core
BCW = B * C * W          # 576
NVAR = float(NWIN * NWIN)          # 49
SCALE = float(D) ** -0.5
PCS = [(0, 512), (512, 512), (1024, 512), (1536, 512), (2048, 256)]
NT = NE // 128           # 64 codebook tiles of 128
NJ = NT // 2             # 32 pairs of row-packed tiles


def _build_program(reps=1):
    import sys
    if "/opt/trn_rl_repo" not in sys.path:
        sys.path.insert(0, "/opt/trn_rl_repo")
    import concourse.bass as bass
    import concourse.mybir as mybir
    import concourse.tile as tile
    from concourse import bacc
    import concourse.bass_isa as bass_isa
    from contextlib import ExitStack

    f32 = mybir.dt.float32
    f32r = mybir.dt.float32r
    u32 = mybir.dt.uint32
    AF = mybir.ActivationFunctionType
    ALU = mybir.AluOpType
    AX = mybir.AxisListType
    ROP = bass_isa.ReduceOp

    nc = bacc.Bacc("TRN2", target_bir_lowering=False, debug=False,
                   num_devices=N_CORES)

    # ---------------- dram I/O ----------------
    d_front = nc.dram_tensor("front", [B, C, H, W], f32, kind="ExternalInput")
    d_back = nc.dram_tensor("back", [B, C, H, W], f32, kind="ExternalInput")
    d_xcm = nc.dram_tensor("front_cm", [D, PIX], f32, kind="ExternalInput")
    d_k4 = nc.dram_tensor("k4", [12, NT // 4 * 128], f32r, kind="ExternalInput")
    d_v = nc.dram_tensor("v_sb", [128, NT * 4], f32r, kind="ExternalInput")
    d_band = nc.dram_tensor("band", [H, H], f32, kind="ExternalInput")
    d_w1T = nc.dram_tensor("w1T", [3, 6], f32, kind="ExternalInput")
    d_b1 = nc.dram_tensor("b1", [6], f32, kind="ExternalInput")
    d_w2cT = nc.dram_tensor("w2cT", [6, 3], f32, kind="ExternalInput")
    d_b2c = nc.dram_tensor("b2c", [3], f32, kind="ExternalInput")
    d_qwT = nc.dram_tensor("qwT", [3, 3], f32, kind="ExternalInput")
    d_n1g = nc.dram_tensor("n1g", [3], f32, kind="ExternalInput")
    d_n1b = nc.dram_tensor("n1b", [3], f32, kind="ExternalInput")
    d_mo1T = nc.dram_tensor("mo1T", [3, 6], f32, kind="ExternalInput")
    d_mob1 = nc.dram_tensor("mob1", [6], f32, kind="ExternalInput")
    d_mo2cT = nc.dram_tensor("mo2cT", [6, 3], f32, kind="ExternalInput")
    d_mob2c = nc.dram_tensor("mob2c", [3], f32, kind="ExternalInput")
    d_n2g = nc.dram_tensor("n2g", [3], f32, kind="ExternalInput")
    d_n2b = nc.dram_tensor("n2b", [3], f32, kind="ExternalInput")
    d_fuseA = nc.dram_tensor("fuseT_a", [4, 3], f32, kind="ExternalInput")
    d_cw = nc.dram_tensor("cw", [45], f32, kind="ExternalInput")
    d_cbias = nc.dram_tensor("cbias", [4 * BCW], f32, kind="ExternalInput")
    d_eps3 = nc.dram_tensor("eps3", [3], f32, kind="ExternalInput")
    d_ones33 = nc.dram_tensor("ones33", [3, 3], f32, kind="ExternalInput")

    d_oa = nc.dram_tensor("out_a", [D, PIX], f32, kind="ExternalOutput")
    d_ff = nc.dram_tensor("ff_full", [B, C, H, W], f32, kind="ExternalOutput")

    def hbcw_ap(handle, b):
        """AP over one batch of a [B,C,H,W] dram tensor ordered (h | c, w)."""
        a = handle[:, :, :, :]
        return bass.AP(tensor=a.tensor, offset=a.offset + b * C * H * W,
                       ap=[[W, H], [H * W, C], [1, W]])

    def col_ap(handle, n):
        """[n] dram vector viewed as [n, 1] (one element per partition)."""
        a = handle[:]
        return bass.AP(tensor=a.tensor, offset=a.offset, ap=[[1, n], [0, 1]])

    def bcast_ap(handle, n):
        """[n] dram vector broadcast across 96 partitions -> [96, n]."""
        a = handle[:]
        return bass.AP(tensor=a.tensor, offset=a.offset, ap=[[0, 96], [1, n]])

    with tile.TileContext(nc) as tc, ExitStack() as ctx:
        consts = ctx.enter_context(tc.tile_pool(name="consts", bufs=1))
        psf = ctx.enter_context(tc.tile_pool(name="psf", bufs=1))
        psft = ctx.enter_context(tc.tile_pool(name="psft", bufs=2))
        mlp = ctx.enter_context(tc.tile_pool(name="mlp", bufs=1))
        attn = ctx.enter_context(tc.tile_pool(name="attn", bufs=3))

        # ---------------- constants to SBUF ----------------
        k4_sb = consts.tile([99, NT // 4 * 128], f32r)
        k_eng = [nc.sync, nc.scalar, nc.gpsimd, nc.sync]
        for g in range(4):
            k_eng[g].dma_start(out=k4_sb[32 * g:32 * g + 3, :],
                               in_=d_k4[3 * g:3 * g + 3, :])
        v_sb = consts.tile([128, NT * 4], f32r)
        nc.sync.dma_start(out=v_sb[:, 0:NT * 2], in_=d_v[:, 0:NT * 2])
        nc.scalar.dma_start(out=v_sb[:, NT * 2:], in_=d_v[:, NT * 2:])
        v_sb4 = v_sb.rearrange("p (n f) -> p n f", f=4)
        band_sb = consts.tile([H, H], f32)
        nc.sync.dma_start(out=band_sb, in_=d_band[:, :])
        w1T_sb = consts.tile([3, 6], f32)
        nc.sync.dma_start(out=w1T_sb, in_=d_w1T[:, :])
        w2cT_sb = consts.tile([6, 3], f32)
        nc.sync.dma_start(out=w2cT_sb, in_=d_w2cT[:, :])
        qwT_sb = consts.tile([3, 3], f32)
        nc.sync.dma_start(out=qwT_sb, in_=d_qwT[:, :])
        mo1T_sb = consts.tile([3, 6], f32)
        nc.sync.dma_start(out=mo1T_sb, in_=d_mo1T[:, :])
        mo2cT_sb = consts.tile([6, 3], f32)
        nc.sync.dma_start(out=mo2cT_sb, in_=d_mo2cT[:, :])
        fuseA_sb = consts.tile([4, 3], f32)
        nc.sync.dma_start(out=fuseA_sb, in_=d_fuseA[:, :])
        ones33_sb = consts.tile([3, 3], f32)
        nc.sync.dma_start(out=ones33_sb, in_=d_ones33[:, :])
        b1_sb = consts.tile([6, 1], f32)
        nc.sync.dma_start(out=b1_sb, in_=col_ap(d_b1, 6))
        b2c_sb = consts.tile([3, 1], f32)
        nc.sync.dma_start(out=b2c_sb, in_=col_ap(d_b2c, 3))
        n1g_sb = consts.tile([3, 1], f32)
        nc.sync.dma_start(out=n1g_sb, in_=col_ap(d_n1g, 3))
        n1b_sb = consts.tile([3, 1], f32)
        nc.sync.dma_start(out=n1b_sb, in_=col_ap(d_n1b, 3))
        mob1_sb = consts.tile([6, 1], f32)
        nc.sync.dma_start(out=mob1_sb, in_=col_ap(d_mob1, 6))
        mob2c_sb = consts.tile([3, 1], f32)
        nc.sync.dma_start(out=mob2c_sb, in_=col_ap(d_mob2c, 3))
        n2g_sb = consts.tile([3, 1], f32)
        nc.sync.dma_start(out=n2g_sb, in_=col_ap(d_n2g, 3))
        n2b_sb = consts.tile([3, 1], f32)
        nc.sync.dma_start(out=n2b_sb, in_=col_ap(d_n2b, 3))
        eps3_sb = consts.tile([3, 1], f32)
        nc.sync.dma_start(out=eps3_sb, in_=col_ap(d_eps3, 3))
        cw_sb = consts.tile([96, 45], f32)
        nc.gpsimd.dma_start(out=cw_sb, in_=bcast_ap(d_cw, 45))
        cbias_sb = consts.tile([96, 4 * BCW], f32)
        nc.gpsimd.dma_start(out=cbias_sb, in_=bcast_ap(d_cbias, 4 * BCW))

        def gelu_exp(dst, x_ps, bias_sb, nparts, tA, tB, tC, tD, tE):
            """dst = gelu_tanh(x_ps + bias) via exp (stays in the nle
            activation-table set): gelu(x) = x * e/(1+e),
            e = exp(2*0.7978845608*(x + 0.044715*x^3))."""
            xg = mlp.tile([nparts, PIX], f32, tag=tA)
            nc.vector.tensor_scalar(xg, x_ps, bias_sb, None, ALU.add)
            t = mlp.tile([nparts, PIX], f32, tag=tB)
            nc.gpsimd.tensor_mul(t, xg, xg)
            nc.gpsimd.tensor_mul(t, t, xg)
            nc.gpsimd.tensor_scalar_mul(t, t, 0.044715)
            nc.gpsimd.tensor_add(t, t, xg)
            e = mlp.tile([nparts, PIX], f32, tag=tC)
            nc.scalar.activation(e, t, AF.Exp, scale=1.5957691216057308)
            d = mlp.tile([nparts, PIX], f32, tag=tD)
            nc.gpsimd.tensor_scalar_add(d, e, 1.0)
            r = mlp.tile([nparts, PIX], f32, tag=tE)
            nc.vector.reciprocal(r, d)
            nc.vector.tensor_mul(r, e, r)
            nc.vector.tensor_mul(dst, xg, r)

        for _rep in range(reps):
            # =========================================================
            # PSF branch (full images, replicated on every core)
            # layout L0: [h=96 partitions | (b, c, w)]
            # =========================================================
            fr = psf.tile([96, B, C, WP], f32, tag="ps11")
            bk = psf.tile([96, B, C, WP], f32)
            nc.vector.memset(fr, 0.0)
            nc.vector.memset(bk, 0.0)
            eng_cycle = [nc.sync, nc.scalar, nc.gpsimd, nc.sync]
            for b in range(B):
                eng_cycle[b].dma_start(out=fr[:, b, :, PAD:PAD + W],
                                       in_=hbcw_ap(d_front, b))
                eng_cycle[2 + b].dma_start(out=bk[:, b, :, PAD:PAD + W],
                                           in_=hbcw_ap(d_back, b))
            sqf = psf.tile([96, B, C, WP], f32, tag="ps1")
            sqb = psf.tile([96, B, C, WP], f32, tag="ps2")
            nc.gpsimd.tensor_mul(sqf, fr, fr)
            nc.gpsimd.tensor_mul(sqb, bk, bk)

            def hbox(dst, src):
                """dst[96,B,C,W] = sum_j src[96,B,C,j:j+W] (7-tap along w)."""
                nc.vector.tensor_add(dst, src[:, :, :, 0:W], src[:, :, :, 1:1 + W])
                for j in range(2, NWIN):
                    nc.vector.tensor_add(dst, dst, src[:, :, :, j:j + W])

            brh_f = psf.tile([96, B, C, W], f32, tag="ps3")
            brh_f2 = psf.tile([96, B, C, W], f32, tag="ps4")
            brh_b = psf.tile([96, B, C, W], f32, tag="ps5")
            brh_b2 = psf.tile([96, B, C, W], f32, tag="ps6")
            hbox(brh_f, fr)
            hbox(brh_f2, sqf)
            hbox(brh_b, bk)
            hbox(brh_b2, sqb)

            s1f = psf.tile([96, B, C, W], f32, tag="ps7")
            s2f = psf.tile([96, B, C, W], f32, tag="ps8")
            s1b = psf.tile([96, B, C, W], f32, tag="ps9")
            s2b = psf.tile([96, B, C, W], f32, tag="ps10")

            with tc.tile_pool(name="ps_vbox", bufs=1, space="PSUM") as ps_vbox, \
                 tc.tile_pool(name="ps_big_a", bufs=1, space="PSUM") as psA:

                def vbox(dst, src):
                    """dst = Band^T @ src over the h (partition) axis."""
                    sflat = src.rearrange("p b c w -> p (b c w)")
                    dflat = dst.rearrange("p b c w -> p (b c w)")
                    for half in range(2):
                        sl = slice(half * 288, half * 288 + 288)
                        bp = ps_vbox.tile([96, 288], f32, tag="vbox_ps", bufs=1)
                        nc.tensor.matmul(bp, band_sb, sflat[:, sl],
                                         start=True, stop=True)
                        nc.vector.tensor_copy(dflat[:, sl], bp)

                vbox(s1f, brh_f)
                vbox(s2f, brh_f2)
                vbox(s1b, brh_b)
                vbox(s2b, brh_b2)

                # ---- mlp_in + ln1 + q (channel-major [d | pix]) ----
                X_sb = mlp.tile([D, PIX], f32, tag="slotA")
                nc.sync.dma_start(out=X_sb, in_=d_xcm[:, :])
                h_sb = mlp.tile([6, PIX], f32, tag="slotB")
                y_sb = mlp.tile([3, PIX], f32, tag="slotE")
                sq_sb = mlp.tile([3, PIX], f32, tag="slotC")
                l_sb = mlp.tile([3, PIX], f32, tag="slotF")
                r3_sb = mlp.tile([3, PIX], f32, tag="slotG")
                x2_sb = mlp.tile([3, PIX], f32, tag="slotB2")
                MCS = [(i * 512, min(512, PIX - i * 512)) for i in range(5)]

                # chunked head pipeline: the attention loop's first pixel
                # chunk only needs q4[:, 0:512], so run the whole chain per
                # 512-pixel chunk to start the exp loop early.
                q4_sb = mlp.tile([99, PIX], f32r, tag="slotQ")
                for off, n in MCS:
                    sl = slice(off, off + n)
                    h_ps = psA.tile([6, 512], f32, tag="hps", bufs=2)
                    nc.tensor.matmul(h_ps[:, 0:n], w1T_sb, X_sb[:, sl],
                                     start=True, stop=True)
                    nc.scalar.activation(h_sb[:, sl], h_ps[:, 0:n],
                                         AF.Gelu_apprx_tanh, bias=b1_sb)
                    y_ps = psA.tile([3, 512], f32, tag="yps", bufs=2)
                    nc.tensor.matmul(y_ps[:, 0:n], w2cT_sb, h_sb[:, sl],
                                     start=True, stop=True)
                    nc.vector.tensor_scalar(y_sb[:, sl], y_ps[:, 0:n],
                                            b2c_sb, None, ALU.add)
                    nc.vector.tensor_mul(sq_sb[:, sl], y_sb[:, sl], y_sb[:, sl])
                    v_ps = psA.tile([3, 512], f32, tag="vps", bufs=1)
                    nc.tensor.matmul(v_ps[:, 0:n], ones33_sb, sq_sb[:, sl],
                                     start=True, stop=True)
                    nc.scalar.activation(l_sb[:, sl], v_ps[:, 0:n],
                                         AF.Ln, bias=eps3_sb)
                    nc.scalar.activation(r3_sb[:, sl], l_sb[:, sl],
                                         AF.Exp, scale=-0.5)
                    nc.vector.tensor_mul(x2_sb[:, sl], y_sb[:, sl], r3_sb[:, sl])
                    nc.vector.tensor_scalar(x2_sb[:, sl], x2_sb[:, sl],
                                            n1g_sb, n1b_sb, ALU.mult, ALU.add)
                    q_ps = psA.tile([3, 512], f32, tag="qps", bufs=1)
                    nc.tensor.matmul(q_ps[:, 0:n], qwT_sb, x2_sb[:, sl],
                                     start=True, stop=True)
                    # replicate q at partition groups {0,32,64,96} (row packing)
                    for g in range(4):
                        nc.vector.tensor_copy(q4_sb[32 * g:32 * g + 3, sl],
                                              q_ps[:, 0:n])

            # ---- PSF stats (overlap the attention loop on DVE) ----
            def stats(s1, s2, mean_t, rstd_t, sd_t):
                """mean = s1/49 ; v = (s2 - s1^2/49)/48 ; sd = sqrt(v);
                rstd = 1/(sd + 1e-8)."""
                v_t = psft.tile([96, B, C, W], f32, tag="st_v")
                u_t = psft.tile([96, B, C, W], f32, tag="st_u")
                rs_t = psft.tile([96, B, C, W], f32, tag="st_r")
                nc.vector.tensor_mul(v_t, s1, s1)
                nc.vector.tensor_scalar_mul(v_t, v_t, -1.0 / (NVAR * (NVAR - 1)))
                nc.vector.tensor_scalar_mul(u_t, s2, 1.0 / (NVAR - 1))
                nc.vector.tensor_add(v_t, v_t, u_t)
                # sqrt via exp(0.5*ln(v)) -- stays in the natural_log_exp set
                nc.scalar.activation(rs_t, v_t, AF.Ln)
                nc.scalar.activation(sd_t, rs_t, AF.Exp, scale=0.5)
                nc.vector.tensor_scalar_add(u_t, sd_t, 1e-8)
                nc.vector.reciprocal(rstd_t, u_t)
                nc.vector.tensor_scalar_mul(mean_t, s1, 1.0 / NVAR)

            m_f = psf.tile([96, B, C, W], f32, tag="ps1")
            r_f = psf.tile([96, B, C, W], f32, tag="ps2")
            sd_f = psf.tile([96, B, C, W], f32, tag="ps12")
            stats(s1f, s2f, m_f, r_f, sd_f)
            m_b = psf.tile([96, B, C, W], f32, tag="ps3")
            r_b = psf.tile([96, B, C, W], f32, tag="ps4")
            sd_b = psf.tile([96, B, C, W], f32, tag="ps5")
            stats(s1b, s2b, m_b, r_b, sd_b)

            # mvnorm(front), mvnorm(back), adain
            xnf = psf.tile([96, B, C, W], f32, tag="ps6")
            nc.vector.tensor_sub(xnf, fr[:, :, :, PAD:PAD + W], m_f)
            nc.vector.tensor_mul(xnf, xnf, r_f)
            xnb = psf.tile([96, B, C, W], f32, tag="ps7")
            nc.vector.tensor_sub(xnb, bk[:, :, :, PAD:PAD + W], m_b)
            nc.vector.tensor_mul(xnb, xnb, r_b)
            xad = psf.tile([96, B, C, W], f32, tag="ps8")
            nc.vector.tensor_mul(xad, xnf, sd_b)
            nc.vector.tensor_add(xad, xad, m_b)

            def conv3(dst, src, wbase, bias_idx=None):
                """1x1 conv over c: dst[:,b,co,w] = sum_ci w[co,ci]*src[:,b,ci,w]."""
                for co in range(3):
                    dco = dst[:, :, co, :]
                    t = psft.tile([96, B, W], f32, tag="conv_t")
                    nc.vector.tensor_scalar_mul(
                        dco, src[:, :, 0, :],
                        cw_sb[:, wbase + co * 3:wbase + co * 3 + 1])
                    nc.vector.tensor_scalar_mul(
                        t, src[:, :, 1, :],
                        cw_sb[:, wbase + co * 3 + 1:wbase + co * 3 + 2])
                    nc.vector.tensor_add(dco, dco, t)
                    nc.vector.tensor_scalar_mul(
                        t, src[:, :, 2, :],
                        cw_sb[:, wbase + co * 3 + 2:wbase + co * 3 + 3])
                    nc.vector.tensor_add(dco, dco, t)
                if bias_idx is not None:
                    dflat = dst.rearrange("p b c w -> p (b c w)")
                    nc.vector.tensor_add(
                        dflat, dflat,
                        cbias_sb[:, bias_idx * BCW:(bias_idx + 1) * BCW])

            EE = psf.tile([96, B, C, W], f32, tag="ps9")
            FF = psf.tile([96, B, C, W], f32, tag="ps13")
            GG = psf.tile([96, B, C, W], f32, tag="ps11")
            HH = psf.tile([96, B, C, W], f32, tag="ps14")
            conv3(EE, xad, 0, bias_idx=0)
            conv3(FF, xnf, 9, bias_idx=1)
            conv3(GG, xnb, 18, bias_idx=2)
            conv3(HH, bk[:, :, :, PAD:PAD + W], 27, bias_idx=3)

            # cosine similarity S[h, b, w] = dot * rsqrt(F2*G2)
            dot = psf.tile([96, B, W], f32)
            f2 = psf.tile([96, B, W], f32)
            g2 = psf.tile([96, B, W], f32)
            tmc = psft.tile([96, B, W], f32, tag="cos_t")
            nc.vector.tensor_mul(dot, FF[:, :, 0, :], GG[:, :, 0, :])
            nc.vector.tensor_mul(f2, FF[:, :, 0, :], FF[:, :, 0, :])
            nc.vector.tensor_mul(g2, GG[:, :, 0, :], GG[:, :, 0, :])
            for cc in range(1, 3):
                nc.vector.tensor_mul(tmc, FF[:, :, cc, :], GG[:, :, cc, :])
                nc.vector.tensor_add(dot, dot, tmc)
                nc.vector.tensor_mul(tmc, FF[:, :, cc, :], FF[:, :, cc, :])
                nc.vector.tensor_add(f2, f2, tmc)
                nc.vector.tensor_mul(tmc, GG[:, :, cc, :], GG[:, :, cc, :])
                nc.vector.tensor_add(g2, g2, tmc)
            nc.vector.tensor_mul(f2, f2, g2)          # F2*G2
            rs2 = psf.tile([96, B, W], f32)
            nc.scalar.activation(g2, f2, AF.Ln)
            nc.scalar.activation(f2, g2, AF.Exp, scale=-0.5)   # 1/(Fn*Gn)
            S = psf.tile([96, B, W], f32)
            nc.vector.tensor_mul(S, dot, f2)

            # global min/max per batch: free-dim reduce then gpsimd all-reduce
            # over partitions (min via max of negated values)
            rmx = psf.tile([96, 2], f32)
            rmn_neg = psf.tile([96, 2], f32)
            nS = psf.tile([96, B, W], f32)
            nc.vector.tensor_scalar_mul(nS, S, -1.0)
            nc.vector.tensor_reduce(rmx, S, axis=AX.X, op=ALU.max)
            nc.vector.tensor_reduce(rmn_neg, nS, axis=AX.X, op=ALU.max)
            mx_bc = psf.tile([96, 2], f32)
            nmn_bc = psf.tile([96, 2], f32)
            nc.gpsimd.partition_all_reduce(mx_bc, rmx, 96, ROP.max)
            nc.gpsimd.partition_all_reduce(nmn_bc, rmn_neg, 96, ROP.max)
            dd_bc = psf.tile([96, 2], f32)
            nc.vector.tensor_add(dd_bc, mx_bc, nmn_bc)    # max - min
            ri_bc = psf.tile([96, 2], f32)
            nc.vector.reciprocal(ri_bc, dd_bc)
            Sn = psf.tile([96, B, W], f32)
            for b in range(B):
                # (S + (-min)) * (1/(max-min))
                nc.vector.tensor_scalar(
                    Sn[:, b, :], S[:, b, :],
                    nmn_bc[:, b:b + 1], ri_bc[:, b:b + 1],
                    ALU.add, ALU.mult)

            # fused = HH + Sn*(EE-HH)
            fused = psf.tile([96, B, C, W], f32, tag="ps10")
            nc.vector.tensor_sub(fused, EE, HH)
            for cc in range(3):
                nc.vector.tensor_mul(fused[:, :, cc, :], fused[:, :, cc, :], Sn)
            ff_flat = fused.rearrange("p b c w -> p (b c w)")
            hh_flat = HH.rearrange("p b c w -> p (b c w)")
            nc.vector.tensor_add(ff_flat, ff_flat, hh_flat)

            # ff contribution: conv with fuse_w[:, 3:6] (no bias)
            ffo = psf.tile([96, B, C, W], f32, tag="ps13")
            conv3(ffo, fused, 36, bias_idx=None)
            for b in range(B):
                nc.sync.dma_start(out=hbcw_ap(d_ff, b), in_=ffo[:, b, :, :])

            # =========================================================
            # attention main loop  (codebook-major score tiles,
            # 4 row-group-packed score matmuls per PSUM tile)
            # =========================================================
            att_sb = mlp.tile([3, PIX], f32, tag="slotD")
            z3_sb = attn.tile([3, 512], f32, tag="z3", bufs=1)
            nc.vector.memset(z3_sb, 0.0)
            with tc.tile_pool(name="ps_sc", bufs=3, space="PSUM") as ps_sc, \
                 tc.tile_pool(name="ps_num", bufs=1, space="PSUM") as ps_num:
                for off, n in PCS:
                    num_ps = ps_num.tile([4, 512], f32, tag="num")
                    for j in range(NJ):
                        r0 = 2 * (j % 2)   # alternate row-group pairs per j
                        m = j // 2
                        sc_ps = ps_sc.tile([128, 1024], f32, tag="sc")
                        for g in range(2):
                            r = r0 + g
                            nc.tensor.matmul(
                                sc_ps[:, g * 512:g * 512 + n],
                                k4_sb[32 * r:32 * r + 3, m * 128:(m + 1) * 128],
                                q4_sb[32 * r:32 * r + 3, off:off + n],
                                tile_position=(32 * r, 0),
                                start=True, stop=True)
                        ex_t = attn.tile([128, 1024], f32r, tag="ex")
                        sc_view = sc_ps.rearrange("p (g c) -> p g c", g=2)[:, :, 0:n]
                        nc.scalar.activation(ex_t[:, 0:2 * n], sc_view, AF.Exp)
                        for g in range(2):
                            nt = 2 * j + g
                            nc.tensor.matmul(
                                num_ps[:, 0:n], v_sb4[:, nt, :],
                                ex_t[:, g * n:(g + 1) * n],
                                start=(j == 0 and g == 0),
                                stop=(j == NJ - 1 and g == 1))
                    # epilogue: att = num[0:3] / num[3]
                    num_sb = attn.tile([4, 512], f32, tag="numsb", bufs=2)
                    nc.vector.tensor_copy(num_sb[:, 0:n], num_ps[:, 0:n])
                    den_sb = attn.tile([1, 512], f32, tag="den", bufs=2)
                    nc.sync.dma_start(out=den_sb[:, 0:n], in_=num_sb[3:4, 0:n])
                    nc.vector.reciprocal(z3_sb[0:1, 0:n], den_sb[:, 0:n])
                    r3a_sb = attn.tile([3, 512], f32, tag="r3a", bufs=2)
                    nc.gpsimd.partition_all_reduce(r3a_sb[:, 0:n],
                                                   z3_sb[:, 0:n], 3, ROP.add)
                    nc.vector.tensor_mul(att_sb[:, off:off + n],
                                         num_sb[0:3, 0:n], r3a_sb[:, 0:n])

            # =========================================================
            # mlp_out + ln2 + fused output conv (attention part)
            # =========================================================
            h2_sb = mlp.tile([6, PIX], f32, tag="slotB")
            y2_sb = mlp.tile([3, PIX], f32, tag="slotE")
            sq2_sb = mlp.tile([3, PIX], f32, tag="slotC")
            l2_sb = mlp.tile([3, PIX], f32, tag="slotF")
            r32_sb = mlp.tile([3, PIX], f32, tag="slotG")
            x4_sb = mlp.tile([4, PIX], f32, tag="slotA")
            oa_sb = mlp.tile([3, PIX], f32, tag="slotB2")
            MCS = [(i * 512, min(512, PIX - i * 512)) for i in range(5)]
            with tc.tile_pool(name="ps_big_b", bufs=1, space="PSUM") as psB:
                h2_ps = psB.tile([6, PIX], f32, tag="bigb")
                for off, n in MCS:
                    nc.tensor.matmul(h2_ps[:, off:off + n], mo1T_sb,
                                     att_sb[:, off:off + n], start=True, stop=True)
                nc.scalar.activation(h2_sb, h2_ps, AF.Gelu_apprx_tanh, bias=mob1_sb)

                y2_ps = psB.tile([3, PIX], f32, tag="bigb")
                for off, n in MCS:
                    nc.tensor.matmul(y2_ps[:, off:off + n], mo2cT_sb,
                                     h2_sb[:, off:off + n], start=True, stop=True)
                nc.vector.tensor_scalar(y2_sb, y2_ps, mob2c_sb, None, ALU.add)
                nc.vector.tensor_mul(sq2_sb, y2_sb, y2_sb)

                v2_ps = psB.tile([3, PIX], f32, tag="bigb")
                for off, n in MCS:
                    nc.tensor.matmul(v2_ps[:, off:off + n], ones33_sb,
                                     sq2_sb[:, off:off + n], start=True, stop=True)
                nc.scalar.activation(l2_sb, v2_ps, AF.Ln, bias=eps3_sb)
                nc.scalar.activation(r32_sb, l2_sb, AF.Exp, scale=-0.5)
                nc.vector.memset(x4_sb, 1.0)   # row 3 stays 1 (fuse bias input)
                nc.vector.tensor_mul(x4_sb[0:3, :], y2_sb, r32_sb)
                nc.vector.tensor_scalar(x4_sb[0:3, :], x4_sb[0:3, :],
                                        n2g_sb, n2b_sb, ALU.mult, ALU.add)

                o_ps = psB.tile([3, PIX], f32, tag="bigb")
                for off, n in MCS:
                    nc.tensor.matmul(o_ps[:, off:off + n], fuseA_sb,
                                     x4_sb[:, off:off + n], start=True, stop=True)
                nc.vector.tensor_copy(oa_sb, o_ps)
            nc.sync.dma_start(out=d_oa[:, :], in_=oa_sb)

    nc.compile()
    return nc


_CACHED = {}


def _prepare_in_maps(inputs):
    f = lambda k: np.asarray(inputs[k], np.float32)
    front, back = f("front"), f("back")
    bg = f("bg_embed")                      # [3, 8192]
    q_w, k_w, v_w = f("q_w"), f("k_w"), f("v_w")
    mi_w1, mi_b1 = f("mi_w1"), f("mi_b1")
    mi_w2, mi_b2 = f("mi_w2"), f("mi_b2")
    mo_w1, mo_b1 = f("mo_w1"), f("mo_b1")
    mo_w2, mo_b2 = f("mo_w2"), f("mo_b2")
    n1_g, n1_b, n2_g, n2_b = f("n1_g"), f("n1_b"), f("n2_g"), f("n2_b")
    e_w, e_b = f("e_w"), f("e_b")
    f_w, f_b = f("f_w"), f("f_b")
    g_w, g_b = f("g_w"), f("g_b")
    h_w, h_b = f("h_w"), f("h_b")
    fuse_w, fuse_b = f("fuse_w"), f("fuse_b")

    # ---- host-side weight repacking (tiny, O(n_embed * d)) ----
    kT = (k_w @ bg) * SCALE                                   # [3, NE]
    # row-group-packed k: pair (j, g) -> row group r = 2*(j%2)+g, col m = j//2
    k4 = np.zeros((12, NT // 4 * 128), np.float32)
    for j in range(NJ):
        for g in range(2):
            r = 2 * (j % 2) + g
            m = j // 2
            nt = 2 * j + g
            k4[3 * r:3 * r + 3, m * 128:(m + 1) * 128] = \
                kT[:, nt * 128:(nt + 1) * 128]
    v = bg.T @ v_w.T                                          # [NE, 3]
    v_ext = np.concatenate([v, np.ones((NE, 1), np.float32)], 1)
    v_np = np.ascontiguousarray(
        v_ext.reshape(NT, 128, 4).transpose(1, 0, 2).reshape(128, NT * 4))
    hh, ww = np.meshgrid(np.arange(H), np.arange(H), indexing="ij")
    band = (np.abs(hh - ww) <= PAD).astype(np.float32)
    w2c = mi_w2 - mi_w2.mean(0, keepdims=True)
    b2c = mi_b2 - mi_b2.mean()
    mo2c = mo_w2 - mo_w2.mean(0, keepdims=True)
    mob2c = mo_b2 - mo_b2.mean()
    fuseT_a = np.concatenate([fuse_w[:, 0:3].T, fuse_b[None, :]], 0)
    cw = np.concatenate([e_w.ravel(), f_w.ravel(), g_w.ravel(),
                         h_w.ravel(), fuse_w[:, 3:6].ravel()])
    cbias = np.concatenate(
        [np.tile(np.repeat(bb, W), B) for bb in (e_b, f_b, g_b, h_b)])

    common = dict(
        front=front, back=back,
        k4=k4, v_sb=v_np,
        band=band,
        w1T=np.ascontiguousarray(mi_w1.T), b1=mi_b1,
        w2cT=np.ascontiguousarray(w2c.T), b2c=b2c,
        qwT=np.ascontiguousarray(q_w.T),
        n1g=n1_g, n1b=n1_b,
        mo1T=np.ascontiguousarray(mo_w1.T), mob1=mo_b1,
        mo2cT=np.ascontiguousarray(mo2c.T), mob2c=mob2c,
        n2g=n2_g, n2b=n2_b,
        fuseT_a=np.ascontiguousarray(fuseT_a),
        cw=np.ascontiguousarray(cw, np.float32),
        cbias=np.ascontiguousarray(cbias, np.float32),
        eps3=np.full(3, 1e-5, np.float32),
        ones33=np.full((3, 3), 1.0 / 3.0, np.float32),
    )
    common = {k: np.ascontiguousarray(v2, np.float32)
              for k, v2 in common.items()}

    in_maps = []
    for i in range(N_CORES):
        sl = front[:, :, HSL * i:HSL * (i + 1), :]          # [B,3,12,96]
        xcm = np.ascontiguousarray(
            sl.transpose(1, 0, 2, 3).reshape(D, PIX), np.float32)
        in_maps.append(dict(common, front_cm=xcm))
    return in_maps


def kernel(**inputs):
    import sys
    if "/opt/trn_rl_repo" not in sys.path:
        sys.path.insert(0, "/opt/trn_rl_repo")
    from concourse.bass_utils import run_bass_kernel_spmd

    in_maps = _prepare_in_maps(inputs)

    if "nc" not in _CACHED:
        _CACHED["nc"] = _build_program()
    nc = _CACHED["nc"]

    res = run_bass_kernel_spmd(nc, in_maps, core_ids=list(range(N_CORES)))
    out = np.array(res.results[0]["ff_full"], np.float32)
    for i in range(N_CORES):
        oa = res.results[i]["out_a"].reshape(D, B, HSL, W)
        out[:, :, HSL * i:HSL * (i + 1), :] += oa.transpose(1, 0, 2, 3)
    return out

